# revision 39
# baseline (speedup 1.0000x reference)
"""Trainium2 Bass kernel for a decoder block (LN -> MHA -> LN -> FFN).

Sharding: heads across the 8 cores for attention (2 heads/core), tokens
across cores for dense/LN2/FFN (512 tokens/core), connected by an
AllToAll of the softmax-normalized ctx in bf16 — one collective per
batch; the first overlaps batch-1 attention, the second overlaps the
batch-0 half of the dense/FFN phase.

v3 structural changes vs v2:
- x is cast to bf16 on the host: bn_stats + normalize run at 2x DVE
  rate and x DMA traffic halves (no fp32 x_full at all).
- AV matmul back to feature-major: vtok (with a ones column) is the
  stationary operand, probs tiles stream as moving -> one 512-col
  matmul per (head, k-tile) instead of four 65-col ones (half the PE
  instructions, 1/8 the LDWEIGHTS).  ctx comes out [65, 512] with the
  softmax denominator in partition 64; the per-column normalize runs
  on the idle GpSimd(Pool) engine: DVE reciprocal of the denominator
  row, Pool partition_broadcast, Pool multiply.
- A2A payload is feature-major, so the consumer rebuild is pure DMA
  (no PE transposes, no copies).
- Band masks / memsets move off the DVE onto Pool.
- Phase 2: dense0 -> LN2-0 -> fc0 -> proj-b0 fills the A2A#2 window;
  a2a-gated consumer DMAs ride the gpsimd queue under tile_wait_until
  floors so the scheduler cannot head-block any engine queue on the
  collective (the v2 trace showed a 30us PE stall from exactly that).
- fc / proj / dense weights stream in chunks (bufs=2 rings) to fit
  SBUF alongside the attention working set.
"""

import numpy as np
import ml_dtypes

B, S, D = 2, 2048, 1024
H, DEP = 16, 64
NT = B * S            # 4096 flattened tokens
NCORES = 8
HPC = H // NCORES     # 2 heads per core
TPC = NT // NCORES    # 512 tokens per core
QM = 512              # q-macro / token-macro size

_cache = {}
DEBUG = False


def _build_program():
    from contextlib import ExitStack
    import concourse.bacc as bacc
    import concourse.tile as tile
    import concourse.mybir as mybir
    from concourse.masks import make_identity

    dt = mybir.dt
    AF = mybir.ActivationFunctionType
    OP = mybir.AluOpType

    nc = bacc.Bacc("TRN2", target_bir_lowering=False, debug=False,
                   num_devices=NCORES)

    def din(name, shape, dtype=dt.float32):
        return nc.dram_tensor(name, shape, dtype, kind="ExternalInput").ap()

    x_bf = din("x_bf", [NT, D], dt.bfloat16)
    x_shard = din("x_shard", [TPC, D], dt.bfloat16)
    wqt = din("wqt", [128, 8, 128], dt.bfloat16)
    wkt = din("wkt", [128, 8, 128], dt.bfloat16)
    wvt = din("wvt", [128, 8, 128], dt.bfloat16)
    qb_i = din("qb", [128, 1])
    kb_i = din("kb", [128, 1])
    vb_i = din("vb", [128, 1])
    g1b_i = din("g1b", [128, D], dt.bfloat16)
    b1b_i = din("b1b", [128, D], dt.bfloat16)
    g2b_i = din("g2b", [128, D], dt.bfloat16)
    b2b_i = din("b2b", [128, D], dt.bfloat16)
    dense_wt = din("dense_wt", [4, 128, 8, 256], dt.bfloat16)
    fc_wt = din("fc_wt", [16, 128, 2, 8, 128], dt.bfloat16)
    fcb_i = din("fcb", [128, 32])
    proj_wt = din("proj_wt", [16, 128, 8, 256], dt.bfloat16)
    mask_i = din("mask_tri", [128, 2, 128], dt.bfloat16)
    out_sh = nc.dram_tensor("out_shard", [TPC, D], dt.float32,
                            kind="ExternalOutput").ap()
    if DEBUG:
        dbg = {nm: nc.dram_tensor(f"dbg_{nm}", shp, dtp,
                                  kind="ExternalOutput").ap()
               for nm, shp, dtp in [
                   ("ctxT", [128, 8, TPC], dt.bfloat16),
                   ("xnsh", [128, 4, D], dt.bfloat16),
                   ("hnT", [128, 8, TPC], dt.bfloat16),
                   ("g1", [128, 32, TPC], dt.bfloat16),
                   ("hn", [128, 4, D], dt.bfloat16),
                   ("qT", [128, S], dt.bfloat16),
                   ("kT", [128, S], dt.bfloat16),
                   ("vtok", [128, 32, 130], dt.bfloat16),
                   ("ctxn", [128, QM], dt.bfloat16),
                   ("ht", [128, 2, D], dt.float32),
                   ("cr0", [128, QM], dt.float32),
                   ("cr1", [128, QM], dt.float32),
                   ("recb", [128, QM], dt.float32),
               ]}

    VAR_SCALE = float(D) / float(D - 1)   # ddof=1 correction

    with tile.TileContext(nc) as tc:
        with ExitStack() as es0:
            P0 = lambda *a, **k: es0.enter_context(tc.tile_pool(*a, **k))
            consts = P0(name="consts", bufs=1)
            dram = es0.enter_context(
                tc.tile_pool(name="dram", bufs=1, space="DRAM"))
            ident_bf = consts.tile([128, 128], dt.bfloat16)
            make_identity(nc, ident_bf)
            qb = consts.tile([128, 1], dt.float32)
            kb = consts.tile([128, 1], dt.float32)
            vb = consts.tile([128, 1], dt.float32)
            nc.sync.dma_start(out=qb, in_=qb_i)
            nc.sync.dma_start(out=kb, in_=kb_i)
            nc.sync.dma_start(out=vb, in_=vb_i)
            tri2 = consts.tile([128, 2, 128], dt.bfloat16)
            nc.sync.dma_start(out=tri2, in_=mask_i)
            ones64 = consts.tile([1, 64], dt.float32)
            nc.vector.memset(ones64, 1.0)

            # ---- cross-phase state (alive through both phases) ----
            xnsh_pool = P0(name="xnsh", bufs=1)
            xn_sh = xnsh_pool.tile([128, 4, D], dt.bfloat16)

            # a2a payload: feature-major [dst, head, dep, tok256]
            a2a_in = [dram.tile([NCORES, HPC, DEP, 256], dt.bfloat16,
                                name=f"a2a_in{bb}") for bb in range(2)]
            a2a_out = [dram.tile([NCORES, HPC, DEP, 256], dt.bfloat16,
                                 name=f"a2a_out{bb}") for bb in range(2)]

            # ------- phase 1: LN1 + QKV + attention, per 512-token macro ---
            with ExitStack() as es1:
                P = lambda *a, **k: es1.enter_context(tc.tile_pool(*a, **k))
                xb_pool = P(name="xb", bufs=3)
                xs_pool = P(name="xs", bufs=2)
                st_pool = P(name="stats", bufs=3)
                xnT_pool = P(name="xnT", bufs=1)
                qkT_pool = P(name="qkT", bufs=1)
                v_pool = P(name="vtok", bufs=1)
                wq_pool = P(name="wq", bufs=1)
                ps_tr = P(name="ps_tr", bufs=1, space="PSUM")
                ps_sc = P(name="ps_sc", bufs=2, space="PSUM")
                ps_qk = P(name="ps_qk", bufs=2, space="PSUM")
                ps_av = P(name="ps_av", bufs=2, space="PSUM")
                ps_bc = P(name="ps_bc", bufs=1, space="PSUM")
                pr_pool = P(name="probs", bufs=4)
                cx_pool = P(name="ctxn", bufs=2)
                rb_pool = P(name="recb", bufs=2)
                cr_pool = P(name="crst", bufs=4)

                # xn_T ring: slot m%2 holds macro m's transposed xn
                xn_T = xnT_pool.tile([128, 8, 2, QM], dt.bfloat16)
                # q_T/k_T ring by batch (token index within batch)
                q_T = qkT_pool.tile([128, S], dt.bfloat16)
                k_T = qkT_pool.tile([128, S], dt.bfloat16)
                vtok = v_pool.tile([128, 32, 130], dt.bfloat16)
                nc.vector.memset(vtok[:, :, 64:65], 1.0)
                nc.vector.memset(vtok[:, :, 129:130], 1.0)

                wq_sb = wq_pool.tile([128, 8, 128], dt.bfloat16)
                wk_sb = wq_pool.tile([128, 8, 128], dt.bfloat16)
                wv_sb = wq_pool.tile([128, 8, 128], dt.bfloat16)
                nc.sync.dma_start(out=wq_sb, in_=wqt)
                nc.sync.dma_start(out=wk_sb, in_=wkt)
                nc.sync.dma_start(out=wv_sb, in_=wvt)

                def rsqrt_dve(out_ap, var_ap, n, pool, tagp):
                    """rstd = 1/sqrt(var*VAR_SCALE) entirely on DVE:
                    bit trick + two Newton iterations."""
                    v = pool.tile([128, n], dt.float32, tag=tagp + "v",
                                  bufs=2, name=tagp + "v")
                    nc.vector.tensor_scalar_mul(out=v, in0=var_ap,
                                                scalar1=VAR_SCALE)
                    y = out_ap
                    yi = y.bitcast(dt.int32)
                    nc.vector.tensor_scalar(
                        out=yi, in0=v.bitcast(dt.int32), scalar1=1,
                        scalar2=None, op0=OP.logical_shift_right)
                    nc.vector.tensor_scalar(
                        out=yi, in0=yi, scalar1=-1, scalar2=0x5f3759df,
                        op0=OP.mult, op1=OP.add)
                    t2 = pool.tile([128, n], dt.float32, tag=tagp + "t",
                                   bufs=2, name=tagp + "t")
                    for _ in range(2):
                        nc.vector.tensor_tensor(out=t2, in0=y, in1=y,
                                                op=OP.mult)
                        nc.vector.tensor_tensor(out=t2, in0=t2, in1=v,
                                                op=OP.mult)
                        nc.vector.tensor_scalar(
                            out=t2, in0=t2, scalar1=-0.5, scalar2=1.5,
                            op0=OP.mult, op1=OP.add)
                        nc.vector.tensor_tensor(out=y, in0=y, in1=t2,
                                                op=OP.mult)

                def ln_macro(src, base_row, xpool, xdt, xtag, xbufs):
                    """Stats for 4 consecutive 128-row tiles."""
                    mv4 = st_pool.tile([128, 4, 2], dt.float32, tag="mv4")
                    xts = []
                    for i in range(4):
                        x_t = xpool.tile([128, D], xdt, tag=xtag, bufs=xbufs)
                        r0 = base_row + 128 * i
                        nc.sync.dma_start(out=x_t, in_=src[r0:r0 + 128, :])
                        stats = st_pool.tile([128, 2, 6], dt.float32,
                                             tag="bnst")
                        nc.vector.bn_stats(out=stats[:, 0, :],
                                           in_=x_t[:, 0:512])
                        nc.vector.bn_stats(out=stats[:, 1, :],
                                           in_=x_t[:, 512:1024])
                        nc.vector.bn_aggr(out=mv4[:, i, :], in_=stats)
                        xts.append(x_t)
                    rstd4 = st_pool.tile([128, 4], dt.float32, tag="rstd4")
                    rsqrt_dve(rstd4, mv4[:, :, 1], 4, st_pool, "rsq")
                    return [(xts[i], mv4[:, i, 0:1], rstd4[:, i:i + 1])
                            for i in range(4)]

                def qkv_part(m, which):
                    slot = m % 2
                    tokb = (QM * m) % S
                    tok = slice(tokb, tokb + QM)
                    if which < 2:
                        w_sb, bias, dst = ((wq_sb, qb, q_T),
                                           (wk_sb, kb, k_T))[which]
                        ps = ps_qk.tile([128, QM], dt.float32, tag="qk")
                        for kc in range(8):
                            nc.tensor.matmul(ps, w_sb[:, kc, :],
                                             xn_T[:, kc, slot, :],
                                             start=(kc == 0), stop=(kc == 7))
                        nc.scalar.activation(out=dst[:, tok], in_=ps,
                                             func=AF.Identity, bias=bias,
                                             scale=1.0)
                        return
                    ps = ps_qk.tile([128, QM], dt.float32, tag="qk")
                    for kc in range(8):
                        nc.tensor.matmul(ps, wv_sb[:, kc, :],
                                         xn_T[:, kc, slot, :],
                                         start=(kc == 0), stop=(kc == 7))
                    vst = cx_pool.tile([128, QM], dt.bfloat16, tag="vst",
                                       bufs=1)
                    nc.scalar.activation(out=vst, in_=ps,
                                         func=AF.Identity, bias=vb,
                                         scale=1.0)
                    for half in range(2):
                        pt = ps_tr.tile([128, 2, 128], dt.bfloat16, tag="xtr")
                        for s2 in range(2):
                            s = 2 * half + s2
                            nc.tensor.transpose(
                                pt[:, s2, :], vst[:, 128 * s:128 * (s + 1)],
                                ident_bf)
                        for s2 in range(2):
                            kt_idx = 4 * m + 2 * half + s2
                            nc.scalar.copy(out=vtok[:, kt_idx, 0:64],
                                           in_=pt[:, s2, 0:64])
                            nc.scalar.copy(out=vtok[:, kt_idx, 65:129],
                                           in_=pt[:, s2, 64:128])

                def attention_macro(b, mm, units):
                    q0 = QM * mm            # within-batch token base
                    nkt = 4 * mm + 4
                    done = 0
                    LAG = min(3, nkt - 1)
                    # feature-major ctx accumulators, one bank per head
                    pcs = [ps_av.tile([128, QM], dt.float32, tag="av",
                                      name=f"pc{h}") for h in range(2)]
                    pbs = [None] * nkt

                    def av(j):
                        kt = 16 * b + j
                        for h in range(2):
                            nc.tensor.matmul(
                                pcs[h][0:65, :],
                                vtok[:, kt, 65 * h:65 * (h + 1)],
                                pbs[j][:, h, :],
                                start=(j == 0), stop=(j == nkt - 1))

                    for j in range(nkt):
                        rel = j - 4 * mm
                        lo = 128 * rel if rel > 0 else 0
                        ks = slice(128 * j, 128 * (j + 1))
                        pb = pr_pool.tile([128, 2, QM], dt.bfloat16, tag="pr",
                                          bufs=4)
                        if lo > 0:
                            nc.gpsimd.memset(pb[:, :, 0:lo], 0.0)
                        for h in range(2):
                            hp = slice(64 * h, 64 * (h + 1))
                            ps = ps_sc.tile([128, QM], dt.float32, tag="sc",
                                            bufs=2)
                            nc.tensor.matmul(
                                ps[:, lo:QM], k_T[hp, ks],
                                q_T[hp, q0 + lo:q0 + QM],
                                start=True, stop=True)
                            nc.scalar.activation(out=pb[:, h, lo:QM],
                                                 in_=ps[:, lo:QM],
                                                 func=AF.Exp, scale=0.125)
                        if rel >= 0:
                            nc.gpsimd.tensor_mul(
                                pb[:, :, lo:lo + 128], pb[:, :, lo:lo + 128],
                                tri2)
                        pbs[j] = pb
                        if j >= LAG:
                            av(j - LAG)
                        target = len(units) * (j + 1) // nkt
                        while done < target:
                            units[done]()
                            done += 1
                    for j in range(nkt - LAG, nkt):
                        av(j)
                    while done < len(units):
                        units[done]()
                        done += 1
                    # stage raw ctx (+denoms) to SBUF with fast ACT copies
                    # so the psum banks free immediately; the normalize +
                    # a2a-write chain is DEFERRED into the next macro's
                    # drain units, where its inputs are long ready and no
                    # engine queue ever head-blocks on it.
                    crs = [cr_pool.tile([128, QM], dt.float32, tag="cr",
                                        bufs=4, name=f"cr{h}")
                           for h in range(2)]
                    # denom rows staged separately at partition base 0:
                    # custom-DVE ops (reciprocal_approx_fast) silently
                    # misread operands with a non-zero base partition
                    dns = [st_pool.tile([1, QM], dt.float32, tag="dn",
                                        bufs=4, name=f"dn{h}")
                           for h in range(2)]
                    for h in range(2):
                        nc.scalar.copy(out=crs[h][0:64, :],
                                       in_=pcs[h][0:64, :])
                        nc.scalar.copy(out=dns[h], in_=pcs[h][64:65, :])

                    def finish(b=b, mm=mm, crs=crs, dns=dns):
                        ctxn = cx_pool.tile([128, QM], dt.bfloat16, tag="cn",
                                            bufs=2)
                        for h in range(2):
                            rec = st_pool.tile([1, QM], dt.float32,
                                               tag="rec", bufs=2)
                            nc.vector.reciprocal_approx_fast(
                                out=rec, in_=dns[h])
                            # broadcast across 64 partitions via a 1-row
                            # fp32r matmul (ones64 stationary)
                            bc = ps_bc.tile([128, QM], dt.float32, tag="bc")
                            nc.tensor.matmul(
                                bc[0:64, :], ones64, rec,
                                start=True, stop=True)
                            nc.vector.tensor_mul(
                                ctxn[64 * h:64 * h + 64, :],
                                crs[h][0:64, :],
                                bc[0:64, :])
                        for half in range(2):
                            nc.sync.dma_start(
                                out=a2a_in[b][2 * mm + half].rearrange(
                                    "h d t -> (h d) t"),
                                in_=ctxn[:, 256 * half:256 * (half + 1)])
                        if DEBUG and b == 1 and mm == 3:
                            nc.sync.dma_start(out=dbg["ctxn"], in_=ctxn)
                            nc.sync.dma_start(out=dbg["cr0"], in_=crs[0])
                            nc.sync.dma_start(out=dbg["cr1"], in_=crs[1])
                            nc.sync.dma_start(out=dbg["recb"], in_=recb)
                    return [finish]

                def make_units(m):
                    """DVE-side LN for macro m issued eagerly; returns PE
                    closures (transposes + QKV matmuls) to drain later."""
                    units = []
                    slot = m % 2
                    for i, (x_t, mean, rstd) in enumerate(
                            ln_macro(x_bf, QM * m, xb_pool, dt.bfloat16,
                                     "xt", 5)):
                        xnb = xb_pool.tile([128, D], dt.bfloat16, tag="xnb",
                                           bufs=3)
                        nc.vector.tensor_scalar(out=xnb, in0=x_t, scalar1=mean,
                                                scalar2=rstd, op0=OP.subtract,
                                                op1=OP.mult)
                        for half in range(2):
                            def u_tr(xnb=xnb, i=i, half=half, slot=slot):
                                pt = ps_tr.tile([128, 4, 128], dt.bfloat16,
                                                tag="xtr")
                                for s2 in range(4):
                                    kc = 4 * half + s2
                                    nc.tensor.transpose(
                                        pt[:, s2, :],
                                        xnb[:, 128 * kc:128 * (kc + 1)],
                                        ident_bf)
                                dst = xn_T[:, 4 * half:4 * half + 4, slot,
                                           128 * i:128 * (i + 1)]
                                if half == 0:
                                    nc.scalar.copy(out=dst, in_=pt)
                                else:
                                    nc.vector.tensor_copy(out=dst, in_=pt)
                            units.append(u_tr)
                    for w in range(3):
                        units.append(lambda m=m, w=w: qkv_part(m, w))
                    return units

                for u in make_units(0):
                    u()
                pend = []
                for m in range(8):
                    nxt = (make_units(m + 1) if m < 7 else []) + pend
                    pend = attention_macro(m // 4, m % 4, nxt)
                    if m == 3:
                        def cc0():
                            nc.gpsimd.collective_compute(
                                "AllToAll", mybir.AluOpType.bypass,
                                replica_groups=[list(range(NCORES))],
                                ins=[a2a_in[0].opt()],
                                outs=[a2a_out[0].opt()],
                            )
                        pend = pend + [cc0]
                    if m == 1:
                        # true xn (gamma/beta applied) for own shard
                        g1b = rb_pool.tile([128, D], dt.bfloat16, tag="g1b",
                                           bufs=1)
                        b1b = rb_pool.tile([128, D], dt.bfloat16, tag="b1b",
                                           bufs=1)
                        nc.sync.dma_start(out=g1b, in_=g1b_i)
                        nc.sync.dma_start(out=b1b, in_=b1b_i)
                        for i, (x_t, mean, rstd) in enumerate(
                                ln_macro(x_shard, 0, xs_pool, dt.bfloat16,
                                         "xr", 4)):
                            xr = xs_pool.tile([128, D], dt.bfloat16,
                                              tag="xrn", bufs=2)
                            nc.vector.tensor_scalar(out=xr, in0=x_t,
                                                    scalar1=mean,
                                                    scalar2=rstd,
                                                    op0=OP.subtract,
                                                    op1=OP.mult)
                            nc.vector.tensor_mul(xr, xr, g1b)
                            nc.vector.tensor_add(xn_sh[:, i, :], xr, b1b)

                for u in pend:
                    u()
                nc.gpsimd.collective_compute(
                    "AllToAll", mybir.AluOpType.bypass,
                    replica_groups=[list(range(NCORES))],
                    ins=[a2a_in[1].opt()], outs=[a2a_out[1].opt()],
                )
                if DEBUG:
                    nc.sync.dma_start(out=dbg["qT"], in_=q_T)
                    nc.sync.dma_start(out=dbg["kT"], in_=k_T)
                    nc.sync.dma_start(out=dbg["vtok"], in_=vtok)

            # ---------------- phase 2: dense, LN2, FFN, per batch half ----
            with ExitStack() as es2:
                P2 = lambda *a, **k: es2.enter_context(tc.tile_pool(*a, **k))
                psd = P2(name="psd", bufs=2, space="PSUM")
                psf = P2(name="psf", bufs=2, space="PSUM")
                pse = P2(name="pse", bufs=2, space="PSUM")

                ctxT_pool = P2(name="ctxT", bufs=1)
                ctxT = ctxT_pool.tile([128, 8, TPC], dt.bfloat16)
                hnT_pool = P2(name="hnT", bufs=1)
                hnT = hnT_pool.tile([128, 8, TPC], dt.bfloat16)
                g1_pool = P2(name="g1sb", bufs=1)
                g1 = g1_pool.tile([128, 32, TPC], dt.bfloat16)
                hn_pool = P2(name="hn", bufs=1)
                hn_true = hn_pool.tile([128, 4, D], dt.bfloat16)
                fcw_pool = P2(name="fcw", bufs=6)
                prj_pool = P2(name="prj", bufs=6)
                dw_pool = P2(name="dw", bufs=4)
                c2_pool = P2(name="c2", bufs=1)
                h_pool = P2(name="hh", bufs=1)
                st2_pool = P2(name="st2", bufs=4)
                out_pool = P2(name="outsb", bufs=3)

                g2b = c2_pool.tile([128, D], dt.bfloat16)
                b2b = c2_pool.tile([128, D], dt.bfloat16)
                fcb = c2_pool.tile([128, 32], dt.float32)

                nc.sync.dma_start(out=g2b, in_=g2b_i)
                nc.sync.dma_start(out=b2b, in_=b2b_i)
                nc.sync.dma_start(out=fcb, in_=fcb_i)

                def fc_dma(ch):
                    t = fcw_pool.tile([128, 2, 8, 128], dt.bfloat16,
                                      tag="fcw")
                    nc.sync.dma_start(out=t, in_=fc_wt[ch])
                    return t

                def pw_dma(ch):
                    t = prj_pool.tile([128, 8, 256], dt.bfloat16, tag="pw")
                    nc.sync.dma_start(out=t, in_=proj_wt[ch])
                    return t

                def dw_dma(dh):
                    t = dw_pool.tile([128, 8, 256], dt.bfloat16, tag="dw")
                    nc.sync.dma_start(out=t, in_=dense_wt[dh])
                    return t

                def ctx_dma(hb, floor_ms):
                    with tc.tile_wait_until(floor_ms):
                        for src in range(NCORES):
                            nc.gpsimd.dma_start(
                                out=ctxT[:, src, 256 * hb:256 * (hb + 1)],
                                in_=a2a_out[hb][src].rearrange(
                                    "h d t -> (h d) t"))

                def dense_ln2(hb):
                    # dense: h = attn_out + (xn*g1 + b1 + dense_b)
                    h_t = h_pool.tile([128, 2, D], dt.float32, tag="ht",
                                      bufs=1)
                    dws = [dw_dma(c) for c in range(4)]
                    for dq in range(4):
                        dsl = slice(256 * dq, 256 * (dq + 1))
                        dwt = dws[dq % 4]
                        for tt in range(2):
                            ts = 2 * hb + tt
                            ps = psd.tile([128, 256], dt.float32, tag="dn")
                            for kc in range(8):
                                nc.tensor.matmul(
                                    ps,
                                    ctxT[:, kc, 128 * ts:128 * (ts + 1)],
                                    dwt[:, kc, :],
                                    start=(kc == 0), stop=(kc == 7))
                            nc.vector.tensor_add(h_t[:, tt, dsl], ps,
                                                 xn_sh[:, ts, dsl])


                    # LN2 -> hn_true (fp32, affine) + hnT (bf16, transposed)
                    mv2 = st2_pool.tile([128, 2, 2], dt.float32, tag="mv2")
                    for tt in range(2):
                        stats = st2_pool.tile([128, 2, 6], dt.float32,
                                              tag="bnst2")
                        nc.vector.bn_stats(out=stats[:, 0, :],
                                           in_=h_t[:, tt, 0:512])
                        nc.vector.bn_stats(out=stats[:, 1, :],
                                           in_=h_t[:, tt, 512:1024])
                        nc.vector.bn_aggr(out=mv2[:, tt, :], in_=stats)
                    rstd2 = st2_pool.tile([128, 2], dt.float32, tag="rstd2")
                    rsqrt_dve(rstd2, mv2[:, :, 1], 2, st2_pool, "rsq2")
                    for tt in range(2):
                        ts = 2 * hb + tt
                        hrb = st2_pool.tile([128, D], dt.bfloat16, tag="hrb",
                                            bufs=2)
                        nc.vector.tensor_scalar(out=hrb, in0=h_t[:, tt, :],
                                                scalar1=mv2[:, tt, 0:1],
                                                scalar2=rstd2[:, tt:tt + 1],
                                                op0=OP.subtract, op1=OP.mult)
                        nc.vector.tensor_mul(hn_true[:, ts, :], hrb, g2b)
                        nc.vector.tensor_add(hn_true[:, ts, :],
                                             hn_true[:, ts, :], b2b)
                        for half in range(2):
                            pt = pse.tile([128, 4, 128], dt.bfloat16,
                                          tag="ctr")
                            for s2 in range(4):
                                kc = 4 * half + s2
                                nc.tensor.transpose(
                                    pt[:, s2, :],
                                    hrb[:, 128 * kc:128 * (kc + 1)], ident_bf)
                            dst = hnT[:, 4 * half:4 * half + 4,
                                      128 * ts:128 * (ts + 1)]
                            if (tt + half) % 2 == 0:
                                nc.scalar.copy(out=dst, in_=pt)
                            else:
                                nc.vector.tensor_copy(out=dst, in_=pt)
                    return h_t

                def fc_half(hb):
                    tb = slice(256 * hb, 256 * (hb + 1))
                    fcw = [fc_dma(c) for c in range(6)]
                    for ch in range(16):
                        fct = fcw[ch % 6]
                        for h2 in range(2):
                            ht = 2 * ch + h2
                            ps = psf.tile([128, 256], dt.float32, tag="fc")
                            for kc in range(8):
                                nc.tensor.matmul(ps, fct[:, h2, kc, :],
                                                 hnT[:, kc, tb],
                                                 start=(kc == 0),
                                                 stop=(kc == 7))
                            nc.scalar.activation(out=g1[:, ht, tb], in_=ps,
                                                 func=AF.Gelu,
                                                 bias=fcb[:, ht:ht + 1],
                                                 scale=1.0)
                        if ch < 10:
                            fcw[ch % 6] = fc_dma(ch + 6)

                def proj_half(hb):
                    pws = [pw_dma(c) for c in range(6)]
                    for q4 in range(4):
                        dsl = slice(256 * q4, 256 * (q4 + 1))
                        pss = [psf.tile([128, 256], dt.float32, tag="fc",
                                        name=f"pj{t}") for t in range(2)]
                        for jc in range(4):
                            ch = 4 * q4 + jc
                            pw = pws[ch % 6]
                            for ti in range(2):
                                ts = 2 * hb + ti
                                tsl = slice(128 * ts, 128 * (ts + 1))
                                for j in range(8):
                                    nc.tensor.matmul(
                                        pss[ti], g1[:, 8 * jc + j, tsl],
                                        pw[:, j, :],
                                        start=(jc == 0 and j == 0),
                                        stop=(jc == 3 and j == 7))
                            if ch < 10:
                                pws[ch % 6] = pw_dma(ch + 6)
                        for ti in range(2):
                            ts = 2 * hb + ti
                            tsl = slice(128 * ts, 128 * (ts + 1))
                            osb = out_pool.tile([128, 256], dt.float32,
                                                tag="osb")
                            nc.vector.tensor_add(osb, pss[ti],
                                                 hn_true[:, ts, dsl])
                            nc.sync.dma_start(out=out_sh[tsl, dsl],
                                              in_=osb)

                # ---- schedule: b0 half fills the A2A#2 window ----
                ctx_dma(0, 10.0)
                dense_ln2(0)
                fc_half(0)
                proj_half(0)
                ctx_dma(1, 20.0)
                h_t1 = dense_ln2(1)
                fc_half(1)
                proj_half(1)
                if DEBUG:
                    nc.sync.dma_start(out=dbg["ctxT"], in_=ctxT)
                    nc.sync.dma_start(out=dbg["xnsh"], in_=xn_sh)
                    nc.sync.dma_start(out=dbg["hnT"], in_=hnT)
                    nc.sync.dma_start(out=dbg["g1"], in_=g1)
                    nc.sync.dma_start(out=dbg["hn"], in_=hn_true)
                    nc.sync.dma_start(out=dbg["ht"], in_=h_t1)

    nc.compile()
    return nc


def _np_reference(x, mask, wq_w, wq_b, wk_w, wk_b, wv_w, wv_b, dense_w,
                  dense_b, gamma1, beta1, gamma2, beta2, fc_w, proj_w):
    """Pure-numpy fallback for non-causal masks (never hit in practice)."""
    import math
    erf = np.vectorize(math.erf)

    def ln(x, g, b):
        mu = x.mean(-1, keepdims=True)
        sd = x.std(-1, ddof=1, keepdims=True)
        return g * ((x - mu) / (sd + 1e-6)) + b

    x = x.astype(np.float64)
    xn = ln(x, gamma1, beta1)
    q = (xn @ wq_w.T + wq_b).reshape(B, S, H, DEP).transpose(0, 2, 1, 3)
    k = (xn @ wk_w.T + wk_b).reshape(B, S, H, DEP).transpose(0, 2, 1, 3)
    v = (xn @ wv_w.T + wv_b).reshape(B, S, H, DEP).transpose(0, 2, 1, 3)
    sc = np.einsum("bhqd,bhkd->bhqk", q, k) / np.sqrt(DEP) + mask * -1e9
    sc = sc - sc.max(-1, keepdims=True)
    e = np.exp(sc)
    a = e / e.sum(-1, keepdims=True)
    ctx = np.einsum("bhqk,bhkd->bhqd", a, v).transpose(0, 2, 1, 3).reshape(
        B, S, D)
    h = xn + ctx @ dense_w.T + dense_b
    hn = ln(h, gamma2, beta2)
    t = hn @ fc_w.T
    g = 0.5 * t * (1.0 + erf(t / np.sqrt(2.0)))
    return (hn + g @ proj_w.T).astype(np.float32)


def kernel(**inputs):
    x = np.asarray(inputs["x"], np.float32)
    mask = np.asarray(inputs["mask"], np.float32)

    causal = np.array_equal(mask, np.triu(np.ones((S, S), np.float32), k=1))
    if not causal:
        return _np_reference(**{k: np.asarray(v, np.float64 if
                                              np.asarray(v).dtype != np.int32
                                              else np.int32)
                                for k, v in inputs.items()}).reshape(B, S, D)

    if "nc" not in _cache:
        _cache["nc"] = _build_program()
    nc = _cache["nc"]

    bf16 = ml_dtypes.bfloat16
    g1 = np.asarray(inputs["gamma1"], np.float32)
    b1 = np.asarray(inputs["beta1"], np.float32)
    g2 = np.asarray(inputs["gamma2"], np.float32)
    b2 = np.asarray(inputs["beta2"], np.float32)
    dense_w = np.asarray(inputs["dense_w"], np.float32)
    dense_b = np.asarray(inputs["dense_b"], np.float32)
    fc_w = np.asarray(inputs["fc_w"], np.float32)
    proj_w = np.asarray(inputs["proj_w"], np.float32)

    xf = x.reshape(NT, D)
    x_bf = np.ascontiguousarray(xf).astype(bf16)
    shard_rows = []
    for c in range(NCORES):
        base = 512 * (c // 2) + 256 * (c % 2)
        shard_rows.append(np.concatenate(
            [base + np.arange(256), 2048 + base + np.arange(256)]))
    bcast = lambda v, dtp=np.float32: np.ascontiguousarray(
        np.broadcast_to(v.astype(np.float32), (128, D))).astype(dtp)

    # causal diagonal-band 0/1 multiplicative mask [k2, {h0,h1}, q2]
    tri = np.ones((128, 128), np.float32)
    kk = np.arange(128)[:, None]
    qq = np.arange(128)[None, :]
    tri[kk > qq] = 0.0
    mask_tri = np.ascontiguousarray(
        np.repeat(tri[:, None, :], 2, axis=1)).astype(bf16)

    fc_eff = fc_w * g2[None, :]
    fcb = fc_w @ b2
    # pre-arranged, partition-contiguous weight layouts
    # dense: [dh, p, kc, m] with contraction row = 128*kc + p
    dense_rr = np.ascontiguousarray(
        dense_w.T.reshape(8, 128, 4, 256).transpose(2, 1, 0, 3)).astype(bf16)
    # fc: [ch, p, h2, kc, m]: ht tile = 2*ch + h2 (rows 128*ht..+128 of fc
    # out), contraction row = 128*kc + p
    fc_rr = np.ascontiguousarray(
        fc_eff.reshape(16, 2, 128, 8, 128).transpose(0, 4, 1, 3, 2)).astype(
            bf16)
    # proj: [ch, p, j16, dout]: q4 = ch//2, j = 16*(ch%2)+j16, contraction
    # row = 128*j + p, dout slice = 256*q4..+256
    proj_rr = np.ascontiguousarray(
        proj_w.T.reshape(32, 128, 4, 256).transpose(2, 0, 1, 3).reshape(
            4, 4, 8, 128, 256).transpose(0, 1, 3, 2, 4).reshape(
            16, 128, 8, 256)).astype(bf16)
    in_maps = []
    for c in range(NCORES):
        rows = slice(128 * c, 128 * (c + 1))
        im = {
            "x_bf": x_bf,
            "x_shard": np.ascontiguousarray(xf[shard_rows[c]]).astype(bf16),
            "g1b": bcast(g1, bf16), "b1b": bcast(b1 + dense_b, bf16),
            "g2b": bcast(g2, bf16), "b2b": bcast(b2, bf16),
            "dense_wt": dense_rr,
            "fc_wt": fc_rr,
            "fcb": np.ascontiguousarray(fcb.reshape(32, 128).T),
            "proj_wt": proj_rr,
            "mask_tri": mask_tri,
        }
        for nm, w, bias in (("q", np.asarray(inputs["wq_w"], np.float32),
                             np.asarray(inputs["wq_b"], np.float32)),
                            ("k", np.asarray(inputs["wk_w"], np.float32),
                             np.asarray(inputs["wk_b"], np.float32)),
                            ("v", np.asarray(inputs["wv_w"], np.float32),
                             np.asarray(inputs["wv_b"], np.float32))):
            wslice = w[rows]                     # [128, D]
            im[f"w{nm}t"] = np.ascontiguousarray(
                (wslice * g1[None, :]).T.reshape(8, 128, 128).transpose(
                    1, 0, 2)).astype(bf16)
            im[f"{nm}b"] = (bias[rows] + wslice @ b1).reshape(128, 1)
        in_maps.append(im)

    global _last_in_maps
    _last_in_maps = in_maps
    from concourse import bass_utils
    res = bass_utils.run_bass_kernel_spmd(nc, in_maps,
                                          core_ids=list(range(NCORES)))
    out = np.empty((NT, D), np.float32)
    for c in range(NCORES):
        out[shard_rows[c]] = res.results[c]["out_shard"]
    return out.reshape(B, S, D)
